# revision 1
# baseline (speedup 1.0000x reference)
"""GroupedQueryAttention on 8 trn2 NeuronCores.

Full shapes: q [2,8,4,2048,128], k/v [2,8,1,2048,128] -> out [2,8,4,2048,128]
softmax over S (no 1/sqrt(D) scaling; no max-subtraction needed: |scores| <~ 75
so exp() stays well inside fp32 range).

Sharding: 16 (b,h) kv pairs across 8 cores -> 2 pairs/core, each pair has
G=4 query heads sharing one K/V. Per core: 8 independent 2048x2048x128
attention heads, no communication.

Per-core kernel (all matmuls contract over the 128-partition dim):
  - K^T, Q^T prepared via PE transposes (fp32).
  - scoresT [s_tile=128, l_chunk=512] = KT.T @ QT  (float32r, 1 cyc/row)
  - ACT evicts PSUM->SBUF with Exp, output bf16.
  - PV: outT [d=128, l=512] += V_chunk.T-form matmul (lhsT=V natural, bf16)
  - softmax denominator: DVE bf16 adds (2x mode) over the 16 exp tiles
    (2 split accumulators to shorten the bf16 rounding chain), then GPSIMD
    partition_all_reduce across the s-partitions.
  - normalize outT with DVE reciprocal+mul, PE-transpose back to natural
    [l,d] layout, DMA out.
"""

import numpy as np

D = 128
L = 2048
S = 2048
G = 4  # query heads per kv head
NP = 2  # kv pairs per core
NH = NP * G  # 8 q-heads per core
LC = 512  # l chunk (matmul moving free dim)
NLC = L // LC  # 4
NST = S // 128  # 16 s tiles
NLT = L // 128  # 16 l tiles
NCORES = 8

_CACHE = {}


def _build_nc():
    import concourse.bass as bass
    import concourse.bacc as bacc
    import concourse.bass_isa as bass_isa
    import concourse.mybir as mybir
    import concourse.tile as tile
    from concourse.masks import make_identity

    f32 = mybir.dt.float32
    f32r = mybir.dt.float32r
    bf16 = mybir.dt.bfloat16
    AF = mybir.ActivationFunctionType
    ALU = mybir.AluOpType

    nc = bacc.Bacc("TRN2")
    q = nc.declare_dram_parameter("q", [NH, L, D], f32, isOutput=False)
    k = nc.declare_dram_parameter("k", [NP, S, D], f32, isOutput=False)
    v = nc.declare_dram_parameter("v", [NP, S, D], f32, isOutput=False)
    o = nc.declare_dram_parameter("o", [NH, L, D], f32, isOutput=True)

    with tile.TileContext(nc) as tc:
        with (
            tc.tile_pool(name="const", bufs=1) as constp,
            tc.tile_pool(name="kt", bufs=2) as ktp,
            tc.tile_pool(name="qt", bufs=2) as qtp,
            tc.tile_pool(name="vv", bufs=2) as vvp,
            tc.tile_pool(name="nat", bufs=4) as natp,
            tc.tile_pool(name="pe", bufs=10) as pep,
            tc.tile_pool(name="acc", bufs=16) as accp,
            tc.tile_pool(name="epi", bufs=8) as epip,
            tc.tile_pool(name="onat", bufs=12) as onatp,
            tc.tile_pool(name="psum", bufs=4, space="PSUM") as psump,
        ):
            ident = constp.tile([128, 128], f32, tag="ident")
            make_identity(nc, ident)
            nbias = constp.tile([128, 1], f32, tag="nbias")
            nc.vector.memset(nbias, -64.0)

            for pair in range(NP):
                # ---- K^T [d=128, S] via PE transposes ----
                KT = ktp.tile([128, S], f32r, tag="KT")
                for st in range(NST):
                    knat = natp.tile([128, D], f32, tag="knat")
                    nc.sync.dma_start(
                        out=knat, in_=k[pair, st * 128 : (st + 1) * 128, :]
                    )
                    pt = psump.tile([128, 128], f32, tag="ps")
                    nc.tensor.transpose(pt, knat, ident)
                    nc.vector.tensor_copy(KT[:, st * 128 : (st + 1) * 128], pt)

                # ---- V natural [s-chunk p, st, d], cast to bf16 ----
                vnat = vvp.tile([128, NST, D], f32, tag="vnat")
                nc.sync.dma_start(
                    out=vnat, in_=v[pair].rearrange("(t p) d -> p t d", p=128)
                )
                Vb = vvp.tile([128, NST, D], bf16, tag="Vb")
                nc.vector.tensor_copy(Vb, vnat)

                for g in range(G):
                    h = pair * G + g
                    # ---- Q^T [d=128, L] via PE transposes ----
                    QT = qtp.tile([128, L], f32r, tag="QT")
                    for lt in range(NLT):
                        qnat = natp.tile([128, D], f32, tag="qnat")
                        nc.sync.dma_start(
                            out=qnat, in_=q[h, lt * 128 : (lt + 1) * 128, :]
                        )
                        pt = psump.tile([128, 128], f32, tag="ps")
                        nc.tensor.transpose(pt, qnat, ident)
                        nc.vector.tensor_copy(QT[:, lt * 128 : (lt + 1) * 128], pt)

                    # out^T accumulators, one PSUM bank per l-chunk
                    po = [
                        psump.tile([128, LC], f32, tag="po", name=f"po_{h}_{lc}")
                        for lc in range(NLC)
                    ]
                    # split bf16 denominator accumulators (even/odd st)
                    acc = [
                        [
                            accp.tile(
                                [128, LC], bf16, tag="acc", name=f"acc_{h}_{lc}_{i}"
                            )
                            for i in range(2)
                        ]
                        for lc in range(NLC)
                    ]

                    for st in range(NST):
                        pss = []
                        for lc in range(NLC):
                            ps = psump.tile([128, LC], f32, tag="ps")
                            nc.tensor.matmul(
                                ps,
                                lhsT=KT[:, st * 128 : (st + 1) * 128],
                                rhs=QT[:, lc * LC : (lc + 1) * LC],
                                start=True,
                                stop=True,
                            )
                            pss.append(ps)
                        for lc in range(NLC):
                            pe = pep.tile([128, LC], bf16, tag="pe")
                            # exp(s - 64): constant shift keeps exp in fp32/bf16
                            # range (scores reach ~99; fp32 exp overflows at 88)
                            nc.scalar.activation(pe, pss[lc], AF.Exp, bias=nbias)
                            nc.tensor.matmul(
                                po[lc],
                                lhsT=Vb[:, st, :],
                                rhs=pe,
                                start=(st == 0),
                                stop=(st == NST - 1),
                            )
                            a = acc[lc][st % 2]
                            if st < 2:
                                nc.vector.tensor_copy(a, pe)
                            else:
                                nc.vector.tensor_tensor(
                                    out=a, in0=a, in1=pe, op=ALU.add
                                )

                    for lc in range(NLC):
                        den = epip.tile([128, LC], f32, tag="den")
                        nc.vector.tensor_tensor(
                            out=den, in0=acc[lc][0], in1=acc[lc][1], op=ALU.add
                        )
                        nc.gpsimd.partition_all_reduce(
                            den, den, 128, bass_isa.ReduceOp.add
                        )
                        rec = epip.tile([128, LC], f32, tag="rec")
                        nc.vector.reciprocal(rec, den)
                        oT = epip.tile([128, LC], f32, tag="oT")
                        nc.vector.tensor_tensor(
                            out=oT, in0=po[lc], in1=rec, op=ALU.mult
                        )
                        for j in range(4):
                            ptr = psump.tile([128, 128], f32, tag="ps")
                            nc.tensor.transpose(
                                ptr, oT[:, j * 128 : (j + 1) * 128], ident
                            )
                            onat = onatp.tile([128, 128], f32, tag="onat")
                            nc.vector.tensor_copy(onat, ptr)
                            lt = lc * 4 + j
                            nc.sync.dma_start(
                                out=o[h, lt * 128 : (lt + 1) * 128, :], in_=onat
                            )
    if not nc.is_finalized():
        nc.finalize()
    return nc


def _get_nc():
    if "nc" not in _CACHE:
        _CACHE["nc"] = _build_nc()
    return _CACHE["nc"]


def _run(q, k, v, trace=False, trace_kwargs=None):
    from concourse.bass_utils import run_bass_kernel_spmd

    nc = _get_nc()
    # (b,h) pair index = b*8+h; core c owns pairs 2c, 2c+1
    q6 = np.ascontiguousarray(q.reshape(16, G, L, D))
    k6 = np.ascontiguousarray(k.reshape(16, S, D))
    v6 = np.ascontiguousarray(v.reshape(16, S, D))
    in_maps = []
    for c in range(NCORES):
        sl = slice(2 * c, 2 * c + 2)
        in_maps.append(
            {
                "q": np.ascontiguousarray(q6[sl].reshape(NH, L, D)),
                "k": np.ascontiguousarray(k6[sl]),
                "v": np.ascontiguousarray(v6[sl]),
            }
        )
    kwargs = {}
    if trace:
        kwargs["trace"] = True
        if trace_kwargs:
            kwargs.update(trace_kwargs)
    res = run_bass_kernel_spmd(nc, in_maps, list(range(NCORES)), **kwargs)
    outs = [res.results[c]["o"] for c in range(NCORES)]
    full = np.concatenate(
        [o.reshape(NP, G, L, D) for o in outs], axis=0
    )  # [16, 4, L, D]
    out = full.reshape(2, 8, G, L, D).astype(np.float32)
    return out, res


def kernel(q, k, v):
    q = np.asarray(q, dtype=np.float32)
    k = np.asarray(k, dtype=np.float32)
    v = np.asarray(v, dtype=np.float32)
    out, _ = _run(q, k, v, trace=False)
    return out



# revision 3
# speedup vs baseline: 1.3655x; 1.3655x over previous
"""GroupedQueryAttention on 8 trn2 NeuronCores.

Full shapes: q [2,8,4,2048,128], k/v [2,8,1,2048,128] -> out [2,8,4,2048,128]
softmax over S (no 1/sqrt(D) scaling; constant -64 shift keeps exp in range).

The end-to-end time here is dominated by the axon-tunneled host<->device
transfers, not device compute, so all wire I/O is 16-bit: q/k ship as fp16
(scores need the mantissa; |scores| <~ 100 so fp16 range is fine), v ships
as bf16 (feeds the PV matmul against bf16 probs), output returns as fp16.

Sharding: 16 (b,h) kv pairs across 8 cores -> 2 pairs/core, each pair has
G=4 query heads sharing one K/V. Per core: 8 independent 2048x2048x128
attention heads, no communication.

Per-core kernel (all matmuls contract over the 128-partition dim):
  - K^T, Q^T prepared via PE transposes (fp16).
  - scoresT [s_tile=128, l_chunk=512] = KT.T @ QT (fp16 in, fp32 PSUM)
  - ACT evicts PSUM->SBUF with Exp, output bf16 (exp(s-64) can reach ~1e15,
    needs bf16 range; bf16 probs already proven within the error budget).
  - PV: outT [d=128, l=512] += V.T-form matmul (lhsT=V natural, bf16)
  - softmax denominator: DVE bf16 adds (2x mode) over the 16 exp tiles
    (2 split accumulators to shorten the bf16 rounding chain), then GPSIMD
    partition_all_reduce across the s-partitions.
  - normalize outT with DVE reciprocal+mul, PE-transpose back to natural
    [l,d] layout, DMA out as fp16.
"""

import numpy as np

D = 128
L = 2048
S = 2048
G = 4  # query heads per kv head
NP = 2  # kv pairs per core
NH = NP * G  # 8 q-heads per core
LC = 512  # l chunk (matmul moving free dim)
NLC = L // LC  # 4
NST = S // 128  # 16 s tiles
NLT = L // 128  # 16 l tiles
NCORES = 8

_CACHE = {}


def _build_nc():
    import concourse.bass as bass
    import concourse.bacc as bacc
    import concourse.bass_isa as bass_isa
    import concourse.mybir as mybir
    import concourse.tile as tile
    from concourse.masks import make_identity

    f32 = mybir.dt.float32
    f16 = mybir.dt.float16
    bf16 = mybir.dt.bfloat16
    AF = mybir.ActivationFunctionType
    ALU = mybir.AluOpType

    nc = bacc.Bacc("TRN2")
    q = nc.declare_dram_parameter("q", [NH, L, D], f16, isOutput=False)
    k = nc.declare_dram_parameter("k", [NP, S, D], f16, isOutput=False)
    v = nc.declare_dram_parameter("v", [NP, S, D], bf16, isOutput=False)
    o = nc.declare_dram_parameter("o", [NH, L, D], f16, isOutput=True)

    with tile.TileContext(nc) as tc:
        with (
            tc.tile_pool(name="const", bufs=1) as constp,
            tc.tile_pool(name="kt", bufs=2) as ktp,
            tc.tile_pool(name="qt", bufs=2) as qtp,
            tc.tile_pool(name="vv", bufs=2) as vvp,
            tc.tile_pool(name="nat", bufs=4) as natp,
            tc.tile_pool(name="pe", bufs=10) as pep,
            tc.tile_pool(name="acc", bufs=16) as accp,
            tc.tile_pool(name="epi", bufs=8) as epip,
            tc.tile_pool(name="onat", bufs=12) as onatp,
            tc.tile_pool(name="psum", bufs=4, space="PSUM") as psump,
        ):
            identh = constp.tile([128, 128], f16, tag="identh")
            make_identity(nc, identh)
            identf = constp.tile([128, 128], f32, tag="identf")
            make_identity(nc, identf)
            nbias = constp.tile([128, 1], f32, tag="nbias")
            nc.vector.memset(nbias, -64.0)

            for pair in range(NP):
                # ---- K^T [d=128, S] via PE transposes (fp16) ----
                KT = ktp.tile([128, S], f16, tag="KT")
                for st in range(NST):
                    knat = natp.tile([128, D], f16, tag="knat")
                    nc.sync.dma_start(
                        out=knat, in_=k[pair, st * 128 : (st + 1) * 128, :]
                    )
                    pt = psump.tile([128, 128], f16, tag="ps")
                    nc.tensor.transpose(pt, knat, identh)
                    nc.vector.tensor_copy(KT[:, st * 128 : (st + 1) * 128], pt)

                # ---- V natural [s-chunk p, st, d], already bf16 on the wire ----
                Vb = vvp.tile([128, NST, D], bf16, tag="Vb")
                nc.sync.dma_start(
                    out=Vb, in_=v[pair].rearrange("(t p) d -> p t d", p=128)
                )

                for g in range(G):
                    h = pair * G + g
                    # ---- Q^T [d=128, L] via PE transposes (fp16) ----
                    QT = qtp.tile([128, L], f16, tag="QT")
                    for lt in range(NLT):
                        qnat = natp.tile([128, D], f16, tag="qnat")
                        nc.sync.dma_start(
                            out=qnat, in_=q[h, lt * 128 : (lt + 1) * 128, :]
                        )
                        pt = psump.tile([128, 128], f16, tag="ps")
                        nc.tensor.transpose(pt, qnat, identh)
                        nc.vector.tensor_copy(QT[:, lt * 128 : (lt + 1) * 128], pt)

                    # out^T accumulators, one PSUM bank per l-chunk
                    po = [
                        psump.tile([128, LC], f32, tag="po", name=f"po_{h}_{lc}")
                        for lc in range(NLC)
                    ]
                    # split bf16 denominator accumulators (even/odd st)
                    acc = [
                        [
                            accp.tile(
                                [128, LC], bf16, tag="acc", name=f"acc_{h}_{lc}_{i}"
                            )
                            for i in range(2)
                        ]
                        for lc in range(NLC)
                    ]

                    for st in range(NST):
                        pss = []
                        for lc in range(NLC):
                            ps = psump.tile([128, LC], f32, tag="ps")
                            nc.tensor.matmul(
                                ps,
                                lhsT=KT[:, st * 128 : (st + 1) * 128],
                                rhs=QT[:, lc * LC : (lc + 1) * LC],
                                start=True,
                                stop=True,
                            )
                            pss.append(ps)
                        for lc in range(NLC):
                            pe = pep.tile([128, LC], bf16, tag="pe")
                            # exp(s - 64): constant shift keeps exp in fp32/bf16
                            # range (scores reach ~99; fp32 exp overflows at 88)
                            nc.scalar.activation(pe, pss[lc], AF.Exp, bias=nbias)
                            nc.tensor.matmul(
                                po[lc],
                                lhsT=Vb[:, st, :],
                                rhs=pe,
                                start=(st == 0),
                                stop=(st == NST - 1),
                            )
                            a = acc[lc][st % 2]
                            if st < 2:
                                nc.vector.tensor_copy(a, pe)
                            else:
                                nc.vector.tensor_tensor(
                                    out=a, in0=a, in1=pe, op=ALU.add
                                )

                    for lc in range(NLC):
                        den = epip.tile([128, LC], f32, tag="den")
                        nc.vector.tensor_tensor(
                            out=den, in0=acc[lc][0], in1=acc[lc][1], op=ALU.add
                        )
                        nc.gpsimd.partition_all_reduce(
                            den, den, 128, bass_isa.ReduceOp.add
                        )
                        rec = epip.tile([128, LC], f32, tag="rec")
                        nc.vector.reciprocal(rec, den)
                        oT = epip.tile([128, LC], f32, tag="oT")
                        nc.vector.tensor_tensor(
                            out=oT, in0=po[lc], in1=rec, op=ALU.mult
                        )
                        for j in range(4):
                            ptr = psump.tile([128, 128], f32, tag="ps")
                            nc.tensor.transpose(
                                ptr, oT[:, j * 128 : (j + 1) * 128], identf
                            )
                            onat = onatp.tile([128, 128], f16, tag="onat")
                            nc.vector.tensor_copy(onat, ptr)
                            lt = lc * 4 + j
                            nc.sync.dma_start(
                                out=o[h, lt * 128 : (lt + 1) * 128, :], in_=onat
                            )
    if not nc.is_finalized():
        nc.finalize()
    return nc


def _get_nc():
    if "nc" not in _CACHE:
        _CACHE["nc"] = _build_nc()
    return _CACHE["nc"]


def _run(q, k, v, trace=False, trace_kwargs=None):
    import ml_dtypes
    from concourse.bass_utils import run_bass_kernel_spmd

    nc = _get_nc()
    # (b,h) pair index = b*8+h; core c owns pairs 2c, 2c+1
    qh = np.asarray(q, dtype=np.float16).reshape(16, G, L, D)
    kh = np.asarray(k, dtype=np.float16).reshape(16, S, D)
    vh = np.asarray(v, dtype=ml_dtypes.bfloat16).reshape(16, S, D)
    in_maps = []
    for c in range(NCORES):
        sl = slice(2 * c, 2 * c + 2)
        in_maps.append(
            {
                "q": qh[sl].reshape(NH, L, D),
                "k": kh[sl],
                "v": vh[sl],
            }
        )
    kwargs = {}
    if trace:
        kwargs["trace"] = True
        if trace_kwargs:
            kwargs.update(trace_kwargs)
    res = run_bass_kernel_spmd(nc, in_maps, list(range(NCORES)), **kwargs)
    outs = [res.results[c]["o"] for c in range(NCORES)]
    full = np.concatenate(
        [o.reshape(NP, G, L, D) for o in outs], axis=0
    )  # [16, 4, L, D]
    out = full.reshape(2, 8, G, L, D).astype(np.float32)
    return out, res


def kernel(q, k, v):
    out, _ = _run(q, k, v, trace=False)
    return out


# revision 7
# speedup vs baseline: 2.2573x; 1.6531x over previous
"""GroupedQueryAttention on 8 trn2 NeuronCores.

Full shapes: q [2,8,4,2048,128], k/v [2,8,1,2048,128] -> out [2,8,4,2048,128]
softmax over S (no 1/sqrt(D) scaling; constant -64 shift keeps exp in range).

The end-to-end time here is dominated by the axon-tunneled host<->device
transfers, not device compute, so the warm path is engineered around the
wire:
  - all I/O is 16-bit: q/k ship as fp16 (scores need the mantissa; |scores|
    <~ 100 so fp16 range is fine), v ships as bf16 (feeds the PV matmul
    against bf16 probs), output returns as fp16.
  - q/k/v are packed into ONE uint16 dram tensor (the tunnel charges ~100ms
    of fixed overhead per transferred array); the kernel bitcasts slices.
  - jax's persistent compilation cache is enabled: run_bass_kernel_spmd
    re-creates its jax.jit closure every call, and without the cache each
    warm call re-runs lower+walrus+load (~0.3s).

Sharding: 16 (b,h) kv pairs across 8 cores -> 2 pairs/core, each pair has
G=4 query heads sharing one K/V. Per core: 8 independent 2048x2048x128
attention heads, no communication.

Per-core kernel (all matmuls contract over the 128-partition dim):
  - K^T, Q^T prepared via PE transposes (fp16).
  - scoresT [s_tile=128, l_chunk=512] = KT.T @ QT (fp16 in, fp32 PSUM)
  - ACT evicts PSUM->SBUF with Exp, output bf16 (exp(s-64) can reach ~1e15,
    needs bf16 range; bf16 probs already proven within the error budget).
  - PV: outT [d=128, l=512] += V.T-form matmul (lhsT=V natural, bf16)
  - softmax denominator: DVE bf16 adds (2x mode) over the 16 exp tiles
    (2 split accumulators to shorten the bf16 rounding chain), then GPSIMD
    partition_all_reduce across the s-partitions.
  - normalize outT with DVE reciprocal+mul, PE-transpose back to natural
    [l,d] layout, DMA out as fp16.
"""

import numpy as np

D = 128
L = 2048
S = 2048
G = 4  # query heads per kv head
NP = 2  # kv pairs per core
NH = NP * G  # 8 q-heads per core
LC = 512  # l chunk (matmul moving free dim)
NLC = L // LC  # 4
NST = S // 128  # 16 s tiles
NLT = L // 128  # 16 l tiles
NCORES = 8

# packed per-pair row layout in the single uint16 input tensor
QB = G * L  # 8192 rows of q (4 heads x 2048)
KB = QB + S  # then 2048 rows of k
VB = KB + S  # then 2048 rows of v
PAIR_ROWS = VB  # 12288

_CACHE = {}


def _build_nc():
    import concourse.bass as bass
    import concourse.bacc as bacc
    import concourse.bass_isa as bass_isa
    import concourse.mybir as mybir
    import concourse.tile as tile
    from concourse.masks import make_identity

    f32 = mybir.dt.float32
    f16 = mybir.dt.float16
    bf16 = mybir.dt.bfloat16
    u16 = mybir.dt.uint16
    AF = mybir.ActivationFunctionType
    ALU = mybir.AluOpType

    nc = bacc.Bacc("TRN2")
    x = nc.declare_dram_parameter("x", [NP, PAIR_ROWS, D], u16, isOutput=False)
    o = nc.declare_dram_parameter("o", [NH, L, D], f16, isOutput=True)

    with tile.TileContext(nc) as tc:
        with (
            tc.tile_pool(name="const", bufs=1) as constp,
            tc.tile_pool(name="kt", bufs=2) as ktp,
            tc.tile_pool(name="qt", bufs=2) as qtp,
            tc.tile_pool(name="vv", bufs=2) as vvp,
            tc.tile_pool(name="nat", bufs=4) as natp,
            tc.tile_pool(name="pe", bufs=10) as pep,
            tc.tile_pool(name="acc", bufs=16) as accp,
            tc.tile_pool(name="epi", bufs=8) as epip,
            tc.tile_pool(name="onat", bufs=12) as onatp,
            tc.tile_pool(name="psum", bufs=4, space="PSUM") as psump,
        ):
            identh = constp.tile([128, 128], f16, tag="identh")
            make_identity(nc, identh)
            identf = constp.tile([128, 128], f32, tag="identf")
            make_identity(nc, identf)
            nbias = constp.tile([128, 1], f32, tag="nbias")
            nc.vector.memset(nbias, -64.0)

            for pair in range(NP):
                # ---- K^T [d=128, S] via PE transposes (fp16) ----
                KT = ktp.tile([128, S], f16, tag="KT")
                for st in range(NST):
                    knat = natp.tile([128, D], f16, tag="knat")
                    nc.sync.dma_start(
                        out=knat,
                        in_=x[pair, QB + st * 128 : QB + (st + 1) * 128, :].bitcast(
                            f16
                        ),
                    )
                    pt = psump.tile([128, 128], f16, tag="ps")
                    nc.tensor.transpose(pt, knat, identh)
                    nc.vector.tensor_copy(KT[:, st * 128 : (st + 1) * 128], pt)

                # ---- V natural [s-chunk p, st, d], already bf16 on the wire ----
                Vb = vvp.tile([128, NST, D], bf16, tag="Vb")
                nc.sync.dma_start(
                    out=Vb,
                    in_=x[pair, KB : KB + S, :]
                    .bitcast(bf16)
                    .rearrange("(t p) d -> p t d", p=128),
                )

                for g in range(G):
                    h = pair * G + g
                    # ---- Q^T [d=128, L] via PE transposes (fp16) ----
                    QT = qtp.tile([128, L], f16, tag="QT")
                    qrow = g * L
                    for lt in range(NLT):
                        qnat = natp.tile([128, D], f16, tag="qnat")
                        nc.sync.dma_start(
                            out=qnat,
                            in_=x[
                                pair, qrow + lt * 128 : qrow + (lt + 1) * 128, :
                            ].bitcast(f16),
                        )
                        pt = psump.tile([128, 128], f16, tag="ps")
                        nc.tensor.transpose(pt, qnat, identh)
                        nc.vector.tensor_copy(QT[:, lt * 128 : (lt + 1) * 128], pt)

                    # out^T accumulators, one PSUM bank per l-chunk
                    po = [
                        psump.tile([128, LC], f32, tag="po", name=f"po_{h}_{lc}")
                        for lc in range(NLC)
                    ]
                    # split bf16 denominator accumulators (even/odd st)
                    acc = [
                        [
                            accp.tile(
                                [128, LC], bf16, tag="acc", name=f"acc_{h}_{lc}_{i}"
                            )
                            for i in range(2)
                        ]
                        for lc in range(NLC)
                    ]

                    for st in range(NST):
                        pss = []
                        for lc in range(NLC):
                            ps = psump.tile([128, LC], f32, tag="ps")
                            nc.tensor.matmul(
                                ps,
                                lhsT=KT[:, st * 128 : (st + 1) * 128],
                                rhs=QT[:, lc * LC : (lc + 1) * LC],
                                start=True,
                                stop=True,
                            )
                            pss.append(ps)
                        for lc in range(NLC):
                            pe = pep.tile([128, LC], bf16, tag="pe")
                            # exp(s - 64): constant shift keeps exp in fp32/bf16
                            # range (scores reach ~99; fp32 exp overflows at 88)
                            nc.scalar.activation(pe, pss[lc], AF.Exp, bias=nbias)
                            nc.tensor.matmul(
                                po[lc],
                                lhsT=Vb[:, st, :],
                                rhs=pe,
                                start=(st == 0),
                                stop=(st == NST - 1),
                            )
                            a = acc[lc][st % 2]
                            if st < 2:
                                nc.vector.tensor_copy(a, pe)
                            else:
                                nc.vector.tensor_tensor(
                                    out=a, in0=a, in1=pe, op=ALU.add
                                )

                    for lc in range(NLC):
                        den = epip.tile([128, LC], f32, tag="den")
                        nc.vector.tensor_tensor(
                            out=den, in0=acc[lc][0], in1=acc[lc][1], op=ALU.add
                        )
                        nc.gpsimd.partition_all_reduce(
                            den, den, 128, bass_isa.ReduceOp.add
                        )
                        rec = epip.tile([128, LC], f32, tag="rec")
                        nc.vector.reciprocal(rec, den)
                        oT = epip.tile([128, LC], f32, tag="oT")
                        nc.vector.tensor_tensor(
                            out=oT, in0=po[lc], in1=rec, op=ALU.mult
                        )
                        for j in range(4):
                            ptr = psump.tile([128, 128], f32, tag="ps")
                            nc.tensor.transpose(
                                ptr, oT[:, j * 128 : (j + 1) * 128], identf
                            )
                            onat = onatp.tile([128, 128], f16, tag="onat")
                            nc.vector.tensor_copy(onat, ptr)
                            lt = lc * 4 + j
                            nc.sync.dma_start(
                                out=o[h, lt * 128 : (lt + 1) * 128, :], in_=onat
                            )
    if not nc.is_finalized():
        nc.finalize()
    return nc


def _get_nc():
    if "nc" not in _CACHE:
        _CACHE["nc"] = _build_nc()
    return _CACHE["nc"]


def _enable_compile_cache():
    # run_bass_kernel_spmd re-creates its jax.jit closure every call, so
    # without a persistent cache each warm call re-runs lower+walrus+load
    # (~0.3s). The axon PJRT supports executable (de)serialization, so the
    # disk cache turns that into a hash lookup.
    if "cc" in _CACHE:
        return
    _CACHE["cc"] = True
    try:
        import jax

        jax.config.update("jax_compilation_cache_dir", "/tmp/jax_pjrt_cache")
        jax.config.update("jax_persistent_cache_min_entry_size_bytes", -1)
        jax.config.update("jax_persistent_cache_min_compile_time_secs", 0)
        jax.config.update("jax_persistent_cache_enable_xla_caches", "all")
    except Exception:
        pass


def _run(q, k, v, trace=False, trace_kwargs=None):
    import ml_dtypes
    from concourse.bass_utils import run_bass_kernel_spmd

    _enable_compile_cache()
    nc = _get_nc()
    q = np.asarray(q)
    k = np.asarray(k)
    v = np.asarray(v)
    # (b,h) pair index = b*8+h; core c owns pairs 2c, 2c+1.
    # Pack q(fp16) / k(fp16) / v(bf16) bit patterns into one uint16 tensor;
    # np.copyto fuses the dtype cast with the store into the packed layout.
    xfull = np.empty((16, PAIR_ROWS, D), np.uint16)
    np.copyto(
        xfull[:, :QB, :].view(np.float16), q.reshape(16, QB, D), casting="unsafe"
    )
    np.copyto(
        xfull[:, QB:KB, :].view(np.float16), k.reshape(16, S, D), casting="unsafe"
    )
    np.copyto(
        xfull[:, KB:, :].view(ml_dtypes.bfloat16),
        v.reshape(16, S, D),
        casting="unsafe",
    )
    in_maps = [{"x": xfull[2 * c : 2 * c + 2]} for c in range(NCORES)]
    kwargs = {}
    if trace:
        kwargs["trace"] = True
        if trace_kwargs:
            kwargs.update(trace_kwargs)
    res = run_bass_kernel_spmd(nc, in_maps, list(range(NCORES)), **kwargs)
    # single-pass gather: fp16 -> f32 upcast fused into the per-core copy
    out = np.empty((16, G, L, D), dtype=np.float32)
    for c in range(NCORES):
        out[2 * c : 2 * c + 2] = res.results[c]["o"].reshape(NP, G, L, D)
    return out.reshape(2, 8, G, L, D), res


def kernel(q, k, v):
    out, _ = _run(q, k, v, trace=False)
    return out


# revision 8
# speedup vs baseline: 2.5036x; 1.1091x over previous
"""GroupedQueryAttention on 8 trn2 NeuronCores.

Full shapes: q [2,8,4,2048,128], k/v [2,8,1,2048,128] -> out [2,8,4,2048,128]
softmax over S (no 1/sqrt(D) scaling; constant -64 shift keeps exp in range).

The end-to-end time here is dominated by the axon-tunneled host<->device
transfers, not device compute, so the warm path is engineered around the
wire:
  - all I/O is 16-bit: q/k ship as fp16 (scores need the mantissa; |scores|
    <~ 100 so fp16 range is fine), v ships as bf16 (feeds the PV matmul
    against bf16 probs), output returns as fp16.
  - q/k/v are packed into ONE uint16 dram tensor (the tunnel charges ~100ms
    of fixed overhead per transferred array); the kernel bitcasts slices.
  - jax's persistent compilation cache is enabled: run_bass_kernel_spmd
    re-creates its jax.jit closure every call, and without the cache each
    warm call re-runs lower+walrus+load (~0.3s).

Sharding: 16 (b,h) kv pairs across 8 cores -> 2 pairs/core, each pair has
G=4 query heads sharing one K/V. Per core: 8 independent 2048x2048x128
attention heads, no communication.

Per-core kernel (all matmuls contract over the 128-partition dim):
  - K^T, Q^T prepared via PE transposes (fp16).
  - scoresT [s_tile=128, l_chunk=512] = KT.T @ QT (fp16 in, fp32 PSUM)
  - ACT evicts PSUM->SBUF with Exp, output bf16 (exp(s-64) can reach ~1e15,
    needs bf16 range; bf16 probs already proven within the error budget).
  - PV: outT [d=128, l=512] += V.T-form matmul (lhsT=V natural, bf16)
  - softmax denominator: DVE bf16 adds (2x mode) over the 16 exp tiles
    (2 split accumulators to shorten the bf16 rounding chain), then GPSIMD
    partition_all_reduce across the s-partitions.
  - normalize outT with DVE reciprocal+mul, PE-transpose back to natural
    [l,d] layout, DMA out as fp16.
"""

import numpy as np

D = 128
L = 2048
S = 2048
G = 4  # query heads per kv head
NP = 2  # kv pairs per core
NH = NP * G  # 8 q-heads per core
LC = 512  # l chunk (matmul moving free dim)
NLC = L // LC  # 4
NST = S // 128  # 16 s tiles
NLT = L // 128  # 16 l tiles
NCORES = 8

# packed per-pair row layout in the single uint16 input tensor
QB = G * L  # 8192 rows of q (4 heads x 2048)
KB = QB + S  # then 2048 rows of k
VB = KB + S  # then 2048 rows of v
PAIR_ROWS = VB  # 12288

_CACHE = {}


def _build_nc():
    import concourse.bass as bass
    import concourse.bacc as bacc
    import concourse.bass_isa as bass_isa
    import concourse.mybir as mybir
    import concourse.tile as tile
    from concourse.masks import make_identity

    f32 = mybir.dt.float32
    f16 = mybir.dt.float16
    bf16 = mybir.dt.bfloat16
    u16 = mybir.dt.uint16
    AF = mybir.ActivationFunctionType
    ALU = mybir.AluOpType

    nc = bacc.Bacc("TRN2")
    x = nc.declare_dram_parameter("x", [NP, PAIR_ROWS, D], u16, isOutput=False)
    o = nc.declare_dram_parameter("o", [NH, L, D], f16, isOutput=True)

    with tile.TileContext(nc) as tc:
        with (
            tc.tile_pool(name="const", bufs=1) as constp,
            tc.tile_pool(name="kt", bufs=2) as ktp,
            tc.tile_pool(name="qt", bufs=2) as qtp,
            tc.tile_pool(name="vv", bufs=2) as vvp,
            tc.tile_pool(name="nat", bufs=4) as natp,
            tc.tile_pool(name="pe", bufs=10) as pep,
            tc.tile_pool(name="acc", bufs=16) as accp,
            tc.tile_pool(name="epi", bufs=8) as epip,
            tc.tile_pool(name="onat", bufs=12) as onatp,
            tc.tile_pool(name="psum", bufs=4, space="PSUM") as psump,
        ):
            identh = constp.tile([128, 128], f16, tag="identh")
            make_identity(nc, identh)
            identf = constp.tile([128, 128], f32, tag="identf")
            make_identity(nc, identf)
            nbias = constp.tile([128, 1], f32, tag="nbias")
            nc.vector.memset(nbias, -64.0)

            for pair in range(NP):
                # ---- K^T [d=128, S] via PE transposes (fp16) ----
                KT = ktp.tile([128, S], f16, tag="KT")
                for st in range(NST):
                    knat = natp.tile([128, D], f16, tag="knat")
                    nc.sync.dma_start(
                        out=knat,
                        in_=x[pair, QB + st * 128 : QB + (st + 1) * 128, :].bitcast(
                            f16
                        ),
                    )
                    pt = psump.tile([128, 128], f16, tag="ps")
                    nc.tensor.transpose(pt, knat, identh)
                    nc.vector.tensor_copy(KT[:, st * 128 : (st + 1) * 128], pt)

                # ---- V natural [s-chunk p, st, d], already bf16 on the wire ----
                Vb = vvp.tile([128, NST, D], bf16, tag="Vb")
                nc.sync.dma_start(
                    out=Vb,
                    in_=x[pair, KB : KB + S, :]
                    .bitcast(bf16)
                    .rearrange("(t p) d -> p t d", p=128),
                )

                for g in range(G):
                    h = pair * G + g
                    # ---- Q^T [d=128, L] via PE transposes (fp16) ----
                    QT = qtp.tile([128, L], f16, tag="QT")
                    qrow = g * L
                    for lt in range(NLT):
                        qnat = natp.tile([128, D], f16, tag="qnat")
                        nc.sync.dma_start(
                            out=qnat,
                            in_=x[
                                pair, qrow + lt * 128 : qrow + (lt + 1) * 128, :
                            ].bitcast(f16),
                        )
                        pt = psump.tile([128, 128], f16, tag="ps")
                        nc.tensor.transpose(pt, qnat, identh)
                        nc.vector.tensor_copy(QT[:, lt * 128 : (lt + 1) * 128], pt)

                    # out^T accumulators, one PSUM bank per l-chunk
                    po = [
                        psump.tile([128, LC], f32, tag="po", name=f"po_{h}_{lc}")
                        for lc in range(NLC)
                    ]
                    # split bf16 denominator accumulators (even/odd st)
                    acc = [
                        [
                            accp.tile(
                                [128, LC], bf16, tag="acc", name=f"acc_{h}_{lc}_{i}"
                            )
                            for i in range(2)
                        ]
                        for lc in range(NLC)
                    ]

                    for st in range(NST):
                        pss = []
                        for lc in range(NLC):
                            ps = psump.tile([128, LC], f32, tag="ps")
                            nc.tensor.matmul(
                                ps,
                                lhsT=KT[:, st * 128 : (st + 1) * 128],
                                rhs=QT[:, lc * LC : (lc + 1) * LC],
                                start=True,
                                stop=True,
                            )
                            pss.append(ps)
                        for lc in range(NLC):
                            pe = pep.tile([128, LC], bf16, tag="pe")
                            # exp(s - 64): constant shift keeps exp in fp32/bf16
                            # range (scores reach ~99; fp32 exp overflows at 88)
                            nc.scalar.activation(pe, pss[lc], AF.Exp, bias=nbias)
                            nc.tensor.matmul(
                                po[lc],
                                lhsT=Vb[:, st, :],
                                rhs=pe,
                                start=(st == 0),
                                stop=(st == NST - 1),
                            )
                            a = acc[lc][st % 2]
                            if st < 2:
                                nc.vector.tensor_copy(a, pe)
                            else:
                                nc.vector.tensor_tensor(
                                    out=a, in0=a, in1=pe, op=ALU.add
                                )

                    for lc in range(NLC):
                        den = epip.tile([128, LC], f32, tag="den")
                        nc.vector.tensor_tensor(
                            out=den, in0=acc[lc][0], in1=acc[lc][1], op=ALU.add
                        )
                        nc.gpsimd.partition_all_reduce(
                            den, den, 128, bass_isa.ReduceOp.add
                        )
                        rec = epip.tile([128, LC], f32, tag="rec")
                        nc.vector.reciprocal(rec, den)
                        oT = epip.tile([128, LC], f32, tag="oT")
                        nc.vector.tensor_tensor(
                            out=oT, in0=po[lc], in1=rec, op=ALU.mult
                        )
                        for j in range(4):
                            ptr = psump.tile([128, 128], f32, tag="ps")
                            nc.tensor.transpose(
                                ptr, oT[:, j * 128 : (j + 1) * 128], identf
                            )
                            onat = onatp.tile([128, 128], f16, tag="onat")
                            nc.vector.tensor_copy(onat, ptr)
                            lt = lc * 4 + j
                            nc.sync.dma_start(
                                out=o[h, lt * 128 : (lt + 1) * 128, :], in_=onat
                            )
    if not nc.is_finalized():
        nc.finalize()
    return nc


def _get_nc():
    if "nc" not in _CACHE:
        _CACHE["nc"] = _build_nc()
    return _CACHE["nc"]


def _enable_compile_cache():
    # run_bass_kernel_spmd re-creates its jax.jit closure every call, so
    # without a persistent cache each warm call re-runs lower+walrus+load
    # (~0.3s). The axon PJRT supports executable (de)serialization, so the
    # disk cache turns that into a hash lookup.
    if "cc" in _CACHE:
        return
    _CACHE["cc"] = True
    try:
        import jax

        jax.config.update("jax_compilation_cache_dir", "/tmp/jax_pjrt_cache")
        jax.config.update("jax_persistent_cache_min_entry_size_bytes", -1)
        jax.config.update("jax_persistent_cache_min_compile_time_secs", 0)
        jax.config.update("jax_persistent_cache_enable_xla_caches", "all")
    except Exception:
        pass


def _run(q, k, v, trace=False, trace_kwargs=None):
    import ml_dtypes
    from concourse.bass_utils import run_bass_kernel_spmd

    _enable_compile_cache()
    nc = _get_nc()
    q = np.asarray(q)
    k = np.asarray(k)
    v = np.asarray(v)
    # (b,h) pair index = b*8+h; core c owns pairs 2c, 2c+1.
    # Pack q(fp16) / k(fp16) / v(bf16) bit patterns into one uint16 tensor;
    # np.copyto fuses the dtype cast with the store into the packed layout.
    xfull = np.empty((16, PAIR_ROWS, D), np.uint16)
    np.copyto(
        xfull[:, :QB, :].view(np.float16), q.reshape(16, QB, D), casting="unsafe"
    )
    np.copyto(
        xfull[:, QB:KB, :].view(np.float16), k.reshape(16, S, D), casting="unsafe"
    )
    np.copyto(
        xfull[:, KB:, :].view(ml_dtypes.bfloat16),
        v.reshape(16, S, D),
        casting="unsafe",
    )
    in_maps = [{"x": xfull[2 * c : 2 * c + 2]} for c in range(NCORES)]
    kwargs = {}
    if trace:
        kwargs["trace"] = True
        if trace_kwargs:
            kwargs.update(trace_kwargs)
    res = run_bass_kernel_spmd(nc, in_maps, list(range(NCORES)), **kwargs)
    # single-pass gather: fp16 -> f32 upcast fused into the per-core copy
    out = np.empty((16, G, L, D), dtype=np.float32)
    for c in range(NCORES):
        out[2 * c : 2 * c + 2] = res.results[c]["o"].reshape(NP, G, L, D)
    # run_bass_kernel_spmd makes a fresh jit closure per call, so its cache
    # entries are dead on return; left around they accumulate and slow later
    # calls by ~1s. Dropping them costs ~0.08s.
    try:
        import gc

        import jax

        jax.clear_caches()
        gc.collect()
    except Exception:
        pass
    return out.reshape(2, 8, G, L, D), res


def kernel(q, k, v):
    out, _ = _run(q, k, v, trace=False)
    return out
